# revision 32
# baseline (speedup 1.0000x reference)
"""Trainium2 Bass kernel for nn_InstDecoder (segment_reduce + bmm).

Computation (reference semantics):
  1. Per batch b: per-label masked mean of features over voxels
     inst[b, n, c] = mean_{v: labels[b,v]==n+1} features[b, c, v]   (labels 1..100)
  2. pred_kernel = inst @ Wk + bk                                   [B, 100, 64]
  3. pred_masks = pred_kernel @ mask_features.reshape(B, 64, M)     [B, 100, M]

Sharding: data-parallel over B (=2) x 4-way split of the flattened voxel axis
M = D*H*W = 524288 -> 8 cores, each owning a [*, 131072] voxel shard.

Phase 1 (device): per-core partial (sums, counts) over its shard via
one-hot(labels) matmuls accumulated in PSUM -> [65, 104] partials (labels
padded to 104 = 13*8; cols 101..103 are never matched so they stay zero).
The one-hot is built s-major ([128, SUB, 104], label innermost) so the PE
streams contiguous rhs slices. Production is split across engines:
  - most chunks: DVE tensor_tensor is_equal in 2x mode. The label operand is
    pre-repeated 8x on the host (lab8) so BOTH inputs walk innermost step-1
    runs: out[p, s, hi, lo] = (lab8[p, s, lo] == iota104[s, hi, lo]).
  - GPS_CHUNKS: GPSIMD subtract (d = lab - iota) then ACT
    Derivative_Erf(64*d) -- an exact bump: bf16(1.1283792)=1.125 at d==0,
    exactly 0 elsewhere. The 1.125 scale is cancelled by pre-scaling those
    chunks' features (and ones column) by 1/1.125 on the host.
Host: sum partials across the 4 shards of each batch + reciprocal of counts
(tiny glue).

Phase 2 (device): pred_kernel^T = (Wk^T @ sums^T) * (1/counts) + bk  [64,100]
(normalization commutes with the channel contraction), cast bf16, then the
big bmm over the mask_features shard -> [100, 131072] bf16.
PSUM staged as 4 x [100, 1024] 2-bank tiles; each drained by concurrent
ACT/DVE half-copies so banks free in ~0.75us and the PE never idles (stays
at the warm 2.4 GHz clock). mf loads ride the sync HWDGE ring, per-chunk
output stores the gpsimd SWDGE ring, so the streams never block each other.

Features are pre-transposed on the host during sharding (with a ones column
appended for the counts row) so the device kernels need no on-chip transpose
of the bulk data.
"""

import ml_dtypes
import numpy as np

BF16 = ml_dtypes.bfloat16

# ---- problem constants (hardcoded per contract) ----
B = 2
C = 64
KD = 64
D, H, W = 8, 256, 256
M = D * H * W            # 524288 voxels per batch
NUM_MASKS = 100
NL = NUM_MASKS + 1       # labels 0..100 (0 dropped at the end)
NLP = 104                # padded label count (13 * 8) for the 2x-mode one-hot
NSH = 4                  # voxel shards per batch
MSH = M // NSH           # 131072 voxels per core
NCORES = B * NSH

# phase-1 tiling: chunks of [128 partitions, SUB voxel-columns]
P1_SUB = 64
P1_NCH = MSH // (128 * P1_SUB)   # 16 chunks of 8192 voxels
# chunks built by gps-subtract + ACT-bump (rest: DVE is_equal), interleaved
# so production completion order tracks the PE's in-order consumption
GPS_CHUNKS = ()
C0_BF = 1.125                    # bf16(Derivative_Erf table value at 0)

# phase-2 tiling: voxel chunks per DMA load; [100, 2048] psum tiles holding
# one 1024-col group of the even chunk + the same cols of the odd chunk
P2_CHUNK = 8192
P2_NCHU = MSH // P2_CHUNK        # 16
P2_TILE = 1024
P2_NT = P2_CHUNK // P2_TILE      # 8

_STATE = {}

# test.py can set this to a dict; per-phase HW exec times (ns) get stored.
PROFILE = None


def _tile_context(nc):
    """TileContext whose kernel-tail drain splits its semaphore waits into
    one wait_ge instruction each — this container's walrus rejects CTRL
    instructions carrying more than a couple of sync waits."""
    import concourse.tile as tile
    from concourse.vector_clock import ScopedClock

    class _SplitDrainTC(tile.TileContext):
        def _drain_and_barrier(self, tick_clock, wait_clock):
            nc = self.nc
            drain_inst = nc.sync.drain()
            wait_clock.add_sem_waits(
                drain_inst.ins, ScopedClock({None: tick_clock.global_clock}))
            si = drain_inst.ins.sync_info
            waits = list(si.on_wait) if si and si.on_wait else []
            handles = {s.name: s for s in self.sems.allocated().values()}
            if waits:
                si.on_wait = []
                for w in waits:
                    nc.sync.wait_ge(handles[w.ant_name], w.wait_value)
            nc.all_engine_barrier()
            popped = nc._tile_sem_poison_stack.pop()
            assert popped is self._sem_poison
            nc.clear_and_free_semaphores(list(self.sems.allocated().values()))
            nc.all_engine_barrier()

    return _SplitDrainTC(nc)


def _split_excess_waits(nc, max_waits=1):
    """This container's walrus rejects instructions carrying more than
    ~2 semaphore waits. Move excess waits onto same-engine nops inserted
    immediately before the offending instruction (monotonic sems make
    this semantically equivalent)."""
    import bass_rust

    created = {}
    new_names = set()
    for bb in nc.main_func.blocks:
        for ins in bb.instructions:
            if ins.name in new_names:
                continue
            si = ins.sync_info
            if si and si.on_wait and len(si.on_wait) > max_waits:
                waits = list(si.on_wait)
                si.on_wait = waits[:max_waits]
                extra = waits[max_waits:]
                nops = []
                for k in range(0, len(extra), max_waits):
                    n = nc.engines[ins.engine].nop(nofuse=True)
                    n.ins.sync_info = bass_rust.SyncInfo(
                        on_wait=extra[k:k + max_waits], on_update=[])
                    nops.append(n.ins)
                    new_names.add(n.ins.name)
                created[ins.name] = nops
    if not created:
        return
    for bb in nc.main_func.blocks:
        out = []
        for ins in bb.instructions:
            if ins.name in new_names:
                continue
            if ins.name in created:
                out.extend(created[ins.name])
            out.append(ins)
        bb.instructions = out


def _build_phase1():
    import concourse.bass as bass
    import concourse.mybir as mybir
    import concourse.tile as tile

    f32 = mybir.dt.float32
    bf16 = mybir.dt.bfloat16
    SUB = P1_SUB
    nc = bass.Bass()
    ft = nc.declare_dram_parameter("ft", [P1_NCH, 128, SUB * 65], bf16, isOutput=False)
    lab8 = nc.declare_dram_parameter("lab8", [128, P1_NCH * SUB * 8], bf16, isOutput=False)
    labs = nc.declare_dram_parameter("labs", [128, P1_NCH * SUB], bf16, isOutput=False)
    iota = nc.declare_dram_parameter("iota", [128, SUB * NLP], bf16, isOutput=False)
    part = nc.declare_dram_parameter("partials", [65, NLP], f32, isOutput=True)

    with _tile_context(nc) as tc:
        with tc.tile_pool(name="const", bufs=1) as constp, \
             tc.tile_pool(name="io", bufs=3) as iop, \
             tc.tile_pool(name="ohd", bufs=4) as ohdp, \
             tc.tile_pool(name="ohg", bufs=2) as ohgp, \
             tc.tile_pool(name="df", bufs=2) as dfp, \
             tc.tile_pool(name="ps", bufs=1, space="PSUM") as psp, \
             tc.tile_pool(name="out", bufs=1) as outp:
            # constants first so the one-hot producers start early
            lab_t = constp.tile([128, P1_NCH * SUB * 8], bf16)
            nc.sync.dma_start(out=lab_t[:], in_=lab8[:])
            iota_t = constp.tile([128, SUB * NLP], bf16)
            nc.sync.dma_start(out=iota_t[:], in_=iota[:])
            if GPS_CHUNKS:
                # separate tiles per reader: DVE and gps hammering the same
                # tile costs DVE ~66% via SBUF bank conflicts
                labs_t = constp.tile([128, P1_NCH * SUB], bf16)
                nc.sync.dma_start(out=labs_t[:], in_=labs[:])
                iota_g = constp.tile([128, SUB * NLP], bf16)
                nc.sync.dma_start(out=iota_g[:], in_=iota[:])
            # issue every ft load up-front; the 3-slot ring self-throttles
            ftts = []
            for c in range(P1_NCH):
                ftt = iop.tile([128, SUB * 65], bf16, tag="ft")
                nc.sync.dma_start(out=ftt[:], in_=ft[c])
                ftts.append(ftt)
            acc = psp.tile([65, NLP], f32)
            iview = iota_t[:].rearrange("p (s h o) -> p s h o", h=13, o=8)
            gview = iota_g[:].rearrange("p (s l) -> p s l", l=NLP) if GPS_CHUNKS else None
            for c in range(P1_NCH):
                if c not in GPS_CHUNKS:
                    lab_sl = lab_t[:, c * SUB * 8:(c + 1) * SUB * 8] \
                        .rearrange("p (s o) -> p s o", o=8)
                    oht = ohdp.tile([128, SUB * NLP], bf16, tag="ohd")
                    nc.vector.tensor_tensor(
                        out=oht[:].rearrange("p (s h o) -> p s h o", h=13, o=8),
                        in0=lab_sl[:, :, None, :].broadcast_to([128, SUB, 13, 8]),
                        in1=iview,
                        op=mybir.AluOpType.is_equal,
                    )
                else:
                    labsl = labs_t[:, c * SUB:(c + 1) * SUB]
                    dt_ = dfp.tile([128, SUB * NLP], bf16, tag="d")
                    nc.gpsimd.tensor_tensor(
                        out=dt_[:].rearrange("p (s l) -> p s l", l=NLP),
                        in0=labsl[:, :, None].broadcast_to([128, SUB, NLP]),
                        in1=gview,
                        op=mybir.AluOpType.subtract,
                    )
                    oht = ohgp.tile([128, SUB * NLP], bf16, tag="ohg")
                    nc.scalar.activation(
                        out=oht[:], in_=dt_[:],
                        func=mybir.ActivationFunctionType.Derivative_Erf,
                        scale=64.0)
                for j in range(SUB):
                    nc.tensor.matmul(
                        acc[:],
                        lhsT=ftts[c][:, j * 65:(j + 1) * 65],
                        rhs=oht[:, j * NLP:(j + 1) * NLP],
                        start=(c == 0 and j == 0),
                        stop=(c == P1_NCH - 1 and j == SUB - 1),
                    )
            out_t = outp.tile([65, NLP], f32)
            nc.vector.tensor_copy(out=out_t[:], in_=acc[:])
            nc.sync.dma_start(out=part[:], in_=out_t[:])
    _split_excess_waits(nc)
    return nc


def _build_phase2():
    import concourse.bass as bass
    import concourse.mybir as mybir
    import concourse.tile as tile

    f32 = mybir.dt.float32
    bf16 = mybir.dt.bfloat16
    nc = bass.Bass()
    # packed constants: rows 0:65 sums^T [65,101]; cols 101:201 rows 0:64 are
    # 1/counts [64,100]; cols 201:265 rows 0:64 are Wk [64,64]; col 265 is bk
    pcb = nc.declare_dram_parameter("pcb", [128, 266], f32, isOutput=False)
    mf = nc.declare_dram_parameter("mf", [C, MSH], bf16, isOutput=False)
    om = nc.declare_dram_parameter("om", [NUM_MASKS, MSH], bf16, isOutput=True)

    with _tile_context(nc) as tc:
        with tc.tile_pool(name="const", bufs=1) as constp, \
             tc.tile_pool(name="io", bufs=3) as iop, \
             tc.tile_pool(name="ob", bufs=3) as obp, \
             tc.tile_pool(name="ps", bufs=2, space="PSUM") as psp:
            pcb_t = constp.tile([128, 266], f32)
            nc.sync.dma_start(out=pcb_t[:], in_=pcb[:])
            pt_t = pcb_t[0:65, 0:NL]
            cn_t = pcb_t[0:KD, NL:NL + NUM_MASKS]
            wk_t = pcb_t[0:C, 201:201 + KD]
            bk_t = pcb_t[0:KD, 265:266]

            # chunk PAIRS: even chunk on partitions 0:64, odd on 64:128, so
            # interleaved matmuls occupy both PE row-halves (tile_position)
            # -> full-array activity, HAM un-throttles to 2.4 GHz, and the
            # two 64-row matmuls stream concurrently (~194 ns per 512 cols).
            # ALL DMA rides the sync HWDGE ring, loads and stores interleaved
            # in pipeline order so the HBM stream never ping-pongs.
            def load_pair(pr):
                mfp = iop.tile([128, P2_CHUNK], bf16, tag="mf", name=f"mfp{pr}")
                nc.sync.dma_start(
                    out=mfp[0:64, :],
                    in_=mf[:, (2 * pr) * P2_CHUNK:(2 * pr + 1) * P2_CHUNK])
                nc.sync.dma_start(
                    out=mfp[64:128, :],
                    in_=mf[:, (2 * pr + 1) * P2_CHUNK:(2 * pr + 2) * P2_CHUNK])
                return mfp

            NPAIR = P2_NCHU // 2


            # prologue: pkt = (Wk^T @ sums^T) * (1/counts) + bk, cast bf16,
            # replicated onto both partition halves for the two row-groups.
            # (normalizing by counts commutes with the channel contraction)
            pro = psp.tile([NUM_MASKS, 2 * P2_TILE], f32, tag="big")
            pkraw = pro[0:KD, 0:NL]
            nc.tensor.matmul(pkraw, lhsT=wk_t, rhs=pt_t[0:C, :],
                             start=True, stop=True)
            pknorm = constp.tile([KD, NUM_MASKS], f32)
            nc.vector.tensor_tensor(out=pknorm[:], in0=pro[0:KD, 1:NL],
                                    in1=cn_t, op=mybir.AluOpType.mult)
            pkt_sb = constp.tile([KD, NUM_MASKS], f32)
            nc.vector.tensor_scalar_add(out=pkt_sb[:], in0=pknorm[:], scalar1=bk_t)
            pkt2 = constp.tile([128, NUM_MASKS], bf16)
            nc.vector.tensor_copy(out=pkt2[0:KD, :], in_=pkt_sb[:])
            nc.sync.dma_start(out=pkt2[KD:128, :], in_=pkt2[0:KD, :])

            # big bmm: out[100, MSH] = PK^T.T @ mask_features
            # Each psum tile [100, 2048] = even-chunk 1024 cols || odd-chunk
            # 1024 cols; ACT drains the even half, DVE the odd, concurrently.
            mfps = {pr: load_pair(pr) for pr in range(3)}
            for pr in range(NPAIR):
                mfp = mfps.pop(pr)
                ob = obp.tile([NUM_MASKS, 2 * P2_CHUNK], bf16, tag="ob", name=f"ob{pr}")
                for t in range(P2_CHUNK // P2_TILE):
                    ps = psp.tile([NUM_MASKS, 2 * P2_TILE], f32, tag="big")
                    for j in range(P2_TILE // 512):
                        col = t * P2_TILE + j * 512
                        nc.tensor.matmul(
                            ps[:, j * 512:(j + 1) * 512],
                            lhsT=pkt2[0:64, :],
                            rhs=mfp[0:64, col:col + 512],
                            start=True, stop=True, tile_position=(0, 0))
                        nc.tensor.matmul(
                            ps[:, P2_TILE + j * 512:P2_TILE + (j + 1) * 512],
                            lhsT=pkt2[64:128, :],
                            rhs=mfp[64:128, col:col + 512],
                            start=True, stop=True, tile_position=(64, 0))
                    nc.scalar.copy(
                        out=ob[:, t * P2_TILE:(t + 1) * P2_TILE],
                        in_=ps[:, 0:P2_TILE])
                    nc.vector.tensor_copy(
                        out=ob[:, P2_CHUNK + t * P2_TILE:P2_CHUNK + (t + 1) * P2_TILE],
                        in_=ps[:, P2_TILE:2 * P2_TILE])
                # each half stores on its own ring as soon as its producer's
                # copies finish: ACT stores its own (even) half on the scalar
                # HWDGE ring (its copies precede it in FIFO, zero coupling);
                # the DVE (odd) half rides the idle gpsimd ring. Loads keep
                # the sync ring to themselves.
                nc.scalar.dma_start(
                    out=om[:, (2 * pr) * P2_CHUNK:(2 * pr + 1) * P2_CHUNK],
                    in_=ob[:, 0:P2_CHUNK])
                nc.gpsimd.dma_start(
                    out=om[:, (2 * pr + 1) * P2_CHUNK:(2 * pr + 2) * P2_CHUNK],
                    in_=ob[:, P2_CHUNK:2 * P2_CHUNK])
                if pr + 3 < NPAIR:
                    mfps[pr + 3] = load_pair(pr + 3)
    _split_excess_waits(nc)
    return nc


def _get_state():
    if not _STATE:
        _STATE["nc1"] = _build_phase1()
        _STATE["nc2"] = _build_phase2()
    return _STATE


def _run(nc, in_maps, tag):
    import os

    from concourse.bass_utils import run_bass_kernel_spmd

    trace = PROFILE is not None
    kw = {}
    tdir = os.environ.get("BASS_TRACE_DIR")
    if tdir:
        kw["tmpdir"] = os.path.join(tdir, tag)
        os.makedirs(kw["tmpdir"], exist_ok=True)
    res = run_bass_kernel_spmd(nc, in_maps, list(range(NCORES)), trace=trace, **kw)
    if PROFILE is not None:
        PROFILE[tag] = res.exec_time_ns
    return res.results


def kernel(features, mask_features, Wk, bk, init_masks):
    features = np.asarray(features, dtype=np.float32)
    mask_features = np.asarray(mask_features, dtype=np.float32)
    Wk = np.ascontiguousarray(np.asarray(Wk, dtype=np.float32))
    bk = np.asarray(bk, dtype=np.float32)
    init_masks = np.asarray(init_masks)

    st = _get_state()

    # ---- host-side sharding / layout prep ----
    feat = features.reshape(B, C, M)
    ftau = np.empty((B, M, 65), np.float32)
    ftau[:, :, :C] = feat.transpose(0, 2, 1)
    ftau[:, :, C] = 1.0
    # GPS_CHUNKS use the ACT bump one-hot (weight 1.125): pre-divide
    ftau.reshape(B, NSH, P1_NCH, 128 * P1_SUB, 65)[:, :, list(GPS_CHUNKS)] *= (1.0 / C0_BF)
    ftau = ftau.astype(BF16)
    labf = init_masks.reshape(B, M).astype(BF16)
    iota = np.ascontiguousarray(np.broadcast_to(
        np.arange(NLP, dtype=BF16)[None, None, :],
        (128, P1_SUB, NLP)).reshape(128, P1_SUB * NLP))

    in_maps1 = []
    for b in range(B):
        for s in range(NSH):
            sl = slice(s * MSH, (s + 1) * MSH)
            labr = labf[b, sl].reshape(P1_NCH, 128, P1_SUB).transpose(1, 0, 2)
            lab8 = np.ascontiguousarray(np.broadcast_to(
                labr[:, :, :, None], (128, P1_NCH, P1_SUB, 8))
                .reshape(128, P1_NCH * P1_SUB * 8))
            in_maps1.append({
                "ft": ftau[b, sl].reshape(P1_NCH, 128, P1_SUB * 65),
                "lab8": lab8,
                "labs": np.ascontiguousarray(labr.reshape(128, P1_NCH * P1_SUB)),
                "iota": iota,
            })
    r1 = _run(st["nc1"], in_maps1, "phase1")

    # combine shard partials per batch + count reciprocal (tiny glue)
    parts = np.stack([r["partials"] for r in r1]).reshape(B, NSH, 65, NLP).sum(axis=1)
    parts = parts[:, :, :NL]                              # drop pad labels
    cntr = 1.0 / np.maximum(parts[:, 64, 1:NL], 1.0)      # [B, 100]

    # pack sums^T / 1/counts / Wk / bk into one [128, 266] f32 tensor
    pcb = np.zeros((B, 128, 266), np.float32)
    pcb[:, 0:65, 0:NL] = parts
    pcb[:, 0:KD, NL:NL + NUM_MASKS] = cntr[:, None, :]
    pcb[:, 0:C, 201:201 + KD] = Wk[None]
    pcb[:, 0:KD, 265] = bk[None]

    mfr = mask_features.reshape(B, C, M).astype(BF16)
    in_maps2 = []
    for b in range(B):
        for s in range(NSH):
            sl = slice(s * MSH, (s + 1) * MSH)
            in_maps2.append({
                "pcb": pcb[b],
                "mf": mfr[b, :, sl],
            })
    r2 = _run(st["nc2"], in_maps2, "phase2")

    out = np.empty((B, NUM_MASKS, M), np.float32)
    for i in range(NCORES):
        b, s = divmod(i, NSH)
        out[b, :, s * MSH:(s + 1) * MSH] = r2[i]["om"]  # bf16 -> f32 upcast
    return out.reshape(B, NUM_MASKS, D, H, W)


# revision 33
# speedup vs baseline: 1.0672x; 1.0672x over previous
"""Trainium2 Bass kernel for nn_InstDecoder (segment_reduce + bmm).

Computation (reference semantics):
  1. Per batch b: per-label masked mean of features over voxels
     inst[b, n, c] = mean_{v: labels[b,v]==n+1} features[b, c, v]   (labels 1..100)
  2. pred_kernel = inst @ Wk + bk                                   [B, 100, 64]
  3. pred_masks = pred_kernel @ mask_features.reshape(B, 64, M)     [B, 100, M]

Sharding: data-parallel over B (=2) x 4-way split of the flattened voxel axis
M = D*H*W = 524288 -> 8 cores, each owning a [*, 131072] voxel shard.

Phase 1 (device): per-core partial (sums, counts) over its shard via
one-hot(labels) matmuls accumulated in PSUM -> [65, 104] partials (labels
padded to 104 = 13*8; cols 101..103 are never matched so they stay zero).
The one-hot is built s-major ([128, SUB, 104], label innermost) so the PE
streams contiguous rhs slices. Production is split across engines:
  - most chunks: DVE tensor_tensor is_equal in 2x mode. The label operand is
    pre-repeated 8x on the host (lab8) so BOTH inputs walk innermost step-1
    runs: out[p, s, hi, lo] = (lab8[p, s, lo] == iota104[s, hi, lo]).
  - GPS_CHUNKS: GPSIMD subtract (d = lab - iota) then ACT
    Derivative_Erf(64*d) -- an exact bump: bf16(1.1283792)=1.125 at d==0,
    exactly 0 elsewhere. The 1.125 scale is cancelled by pre-scaling those
    chunks' features (and ones column) by 1/1.125 on the host.
Host: sum partials across the 4 shards of each batch + reciprocal of counts
(tiny glue).

Phase 2 (device): pred_kernel^T = (Wk^T @ sums^T) * (1/counts) + bk  [64,100]
(normalization commutes with the channel contraction), cast bf16, then the
big bmm over the mask_features shard -> [100, 131072] bf16.
PSUM staged as 4 x [100, 1024] 2-bank tiles; each drained by concurrent
ACT/DVE half-copies so banks free in ~0.75us and the PE never idles (stays
at the warm 2.4 GHz clock). mf loads ride the sync HWDGE ring, per-chunk
output stores the gpsimd SWDGE ring, so the streams never block each other.

Features are pre-transposed on the host during sharding (with a ones column
appended for the counts row) so the device kernels need no on-chip transpose
of the bulk data.
"""

import ml_dtypes
import numpy as np

BF16 = ml_dtypes.bfloat16

# ---- problem constants (hardcoded per contract) ----
B = 2
C = 64
KD = 64
D, H, W = 8, 256, 256
M = D * H * W            # 524288 voxels per batch
NUM_MASKS = 100
NL = NUM_MASKS + 1       # labels 0..100 (0 dropped at the end)
NLP = 104                # padded label count (13 * 8) for the 2x-mode one-hot
NSH = 4                  # voxel shards per batch
MSH = M // NSH           # 131072 voxels per core
NCORES = B * NSH

# phase-1 tiling: chunks of [128 partitions, SUB voxel-columns]
P1_SUB = 64
P1_NCH = MSH // (128 * P1_SUB)   # 16 chunks of 8192 voxels
# chunks built by gps-subtract + ACT-bump (rest: DVE is_equal), interleaved
# so production completion order tracks the PE's in-order consumption
GPS_CHUNKS = ()
C0_BF = 1.125                    # bf16(Derivative_Erf table value at 0)

# phase-2 tiling: voxel chunks per DMA load; [100, 2048] psum tiles holding
# one 1024-col group of the even chunk + the same cols of the odd chunk
P2_CHUNK = 8192
P2_NCHU = MSH // P2_CHUNK        # 16
P2_TILE = 1024
P2_NT = P2_CHUNK // P2_TILE      # 8

_STATE = {}

# test.py can set this to a dict; per-phase HW exec times (ns) get stored.
PROFILE = None


def _tile_context(nc):
    """TileContext whose kernel-tail drain splits its semaphore waits into
    one wait_ge instruction each — this container's walrus rejects CTRL
    instructions carrying more than a couple of sync waits."""
    import concourse.tile as tile
    from concourse.vector_clock import ScopedClock

    class _SplitDrainTC(tile.TileContext):
        def _drain_and_barrier(self, tick_clock, wait_clock):
            nc = self.nc
            drain_inst = nc.sync.drain()
            wait_clock.add_sem_waits(
                drain_inst.ins, ScopedClock({None: tick_clock.global_clock}))
            si = drain_inst.ins.sync_info
            waits = list(si.on_wait) if si and si.on_wait else []
            handles = {s.name: s for s in self.sems.allocated().values()}
            if waits:
                si.on_wait = []
                for w in waits:
                    nc.sync.wait_ge(handles[w.ant_name], w.wait_value)
            nc.all_engine_barrier()
            popped = nc._tile_sem_poison_stack.pop()
            assert popped is self._sem_poison
            nc.clear_and_free_semaphores(list(self.sems.allocated().values()))
            nc.all_engine_barrier()

    return _SplitDrainTC(nc)


def _split_excess_waits(nc, max_waits=1):
    """This container's walrus rejects instructions carrying more than
    ~2 semaphore waits. Move excess waits onto same-engine nops inserted
    immediately before the offending instruction (monotonic sems make
    this semantically equivalent)."""
    import bass_rust

    created = {}
    new_names = set()
    for bb in nc.main_func.blocks:
        for ins in bb.instructions:
            if ins.name in new_names:
                continue
            si = ins.sync_info
            if si and si.on_wait and len(si.on_wait) > max_waits:
                waits = list(si.on_wait)
                si.on_wait = waits[:max_waits]
                extra = waits[max_waits:]
                nops = []
                for k in range(0, len(extra), max_waits):
                    n = nc.engines[ins.engine].nop(nofuse=True)
                    n.ins.sync_info = bass_rust.SyncInfo(
                        on_wait=extra[k:k + max_waits], on_update=[])
                    nops.append(n.ins)
                    new_names.add(n.ins.name)
                created[ins.name] = nops
    if not created:
        return
    for bb in nc.main_func.blocks:
        out = []
        for ins in bb.instructions:
            if ins.name in new_names:
                continue
            if ins.name in created:
                out.extend(created[ins.name])
            out.append(ins)
        bb.instructions = out


def _build_phase1():
    import concourse.bass as bass
    import concourse.mybir as mybir
    import concourse.tile as tile

    f32 = mybir.dt.float32
    bf16 = mybir.dt.bfloat16
    SUB = P1_SUB
    nc = bass.Bass()
    ft = nc.declare_dram_parameter("ft", [P1_NCH, 128, SUB * 65], bf16, isOutput=False)
    lab8 = nc.declare_dram_parameter("lab8", [128, P1_NCH * SUB * 8], bf16, isOutput=False)
    labs = nc.declare_dram_parameter("labs", [128, P1_NCH * SUB], bf16, isOutput=False)
    iota = nc.declare_dram_parameter("iota", [128, SUB * NLP], bf16, isOutput=False)
    part = nc.declare_dram_parameter("partials", [65, NLP], f32, isOutput=True)

    with _tile_context(nc) as tc:
        with tc.tile_pool(name="const", bufs=1) as constp, \
             tc.tile_pool(name="io", bufs=3) as iop, \
             tc.tile_pool(name="ohd", bufs=4) as ohdp, \
             tc.tile_pool(name="ohg", bufs=2) as ohgp, \
             tc.tile_pool(name="df", bufs=2) as dfp, \
             tc.tile_pool(name="ps", bufs=1, space="PSUM") as psp, \
             tc.tile_pool(name="out", bufs=1) as outp:
            # constants first so the one-hot producers start early
            lab_t = constp.tile([128, P1_NCH * SUB * 8], bf16)
            nc.sync.dma_start(out=lab_t[:], in_=lab8[:])
            iota_t = constp.tile([128, SUB * NLP], bf16)
            nc.sync.dma_start(out=iota_t[:], in_=iota[:])
            if GPS_CHUNKS:
                # separate tiles per reader: DVE and gps hammering the same
                # tile costs DVE ~66% via SBUF bank conflicts
                labs_t = constp.tile([128, P1_NCH * SUB], bf16)
                nc.sync.dma_start(out=labs_t[:], in_=labs[:])
                iota_g = constp.tile([128, SUB * NLP], bf16)
                nc.sync.dma_start(out=iota_g[:], in_=iota[:])
            # issue every ft load up-front; the 3-slot ring self-throttles
            ftts = []
            for c in range(P1_NCH):
                ftt = iop.tile([128, SUB * 65], bf16, tag="ft")
                nc.sync.dma_start(out=ftt[:], in_=ft[c])
                ftts.append(ftt)
            acc = psp.tile([65, NLP], f32)
            iview = iota_t[:].rearrange("p (s h o) -> p s h o", h=13, o=8)
            gview = iota_g[:].rearrange("p (s l) -> p s l", l=NLP) if GPS_CHUNKS else None
            for c in range(P1_NCH):
                if c not in GPS_CHUNKS:
                    lab_sl = lab_t[:, c * SUB * 8:(c + 1) * SUB * 8] \
                        .rearrange("p (s o) -> p s o", o=8)
                    oht = ohdp.tile([128, SUB * NLP], bf16, tag="ohd")
                    nc.vector.tensor_tensor(
                        out=oht[:].rearrange("p (s h o) -> p s h o", h=13, o=8),
                        in0=lab_sl[:, :, None, :].broadcast_to([128, SUB, 13, 8]),
                        in1=iview,
                        op=mybir.AluOpType.is_equal,
                    )
                else:
                    labsl = labs_t[:, c * SUB:(c + 1) * SUB]
                    dt_ = dfp.tile([128, SUB * NLP], bf16, tag="d")
                    nc.gpsimd.tensor_tensor(
                        out=dt_[:].rearrange("p (s l) -> p s l", l=NLP),
                        in0=labsl[:, :, None].broadcast_to([128, SUB, NLP]),
                        in1=gview,
                        op=mybir.AluOpType.subtract,
                    )
                    oht = ohgp.tile([128, SUB * NLP], bf16, tag="ohg")
                    nc.scalar.activation(
                        out=oht[:], in_=dt_[:],
                        func=mybir.ActivationFunctionType.Derivative_Erf,
                        scale=64.0)
                for j in range(SUB):
                    nc.tensor.matmul(
                        acc[:],
                        lhsT=ftts[c][:, j * 65:(j + 1) * 65],
                        rhs=oht[:, j * NLP:(j + 1) * NLP],
                        start=(c == 0 and j == 0),
                        stop=(c == P1_NCH - 1 and j == SUB - 1),
                    )
            out_t = outp.tile([65, NLP], f32)
            nc.vector.tensor_copy(out=out_t[:], in_=acc[:])
            nc.sync.dma_start(out=part[:], in_=out_t[:])
    _split_excess_waits(nc)
    return nc


def _build_phase2():
    import concourse.bass as bass
    import concourse.mybir as mybir
    import concourse.tile as tile

    f32 = mybir.dt.float32
    bf16 = mybir.dt.bfloat16
    nc = bass.Bass()
    # packed constants: rows 0:65 sums^T [65,101]; cols 101:201 rows 0:64 are
    # 1/counts [64,100]; cols 201:265 rows 0:64 are Wk [64,64]; col 265 is bk
    pcb = nc.declare_dram_parameter("pcb", [128, 266], f32, isOutput=False)
    mf = nc.declare_dram_parameter("mf", [C, MSH], bf16, isOutput=False)
    om = nc.declare_dram_parameter("om", [NUM_MASKS, MSH], bf16, isOutput=True)

    with _tile_context(nc) as tc:
        with tc.tile_pool(name="const", bufs=1) as constp, \
             tc.tile_pool(name="io", bufs=3) as iop, \
             tc.tile_pool(name="ob", bufs=3) as obp, \
             tc.tile_pool(name="ps", bufs=2, space="PSUM") as psp:
            pcb_t = constp.tile([128, 266], f32)
            nc.sync.dma_start(out=pcb_t[:], in_=pcb[:])
            pt_t = pcb_t[0:65, 0:NL]
            cn_t = pcb_t[0:KD, NL:NL + NUM_MASKS]
            wk_t = pcb_t[0:C, 201:201 + KD]
            bk_t = pcb_t[0:KD, 265:266]

            # chunk PAIRS: even chunk on partitions 0:64, odd on 64:128, so
            # interleaved matmuls occupy both PE row-halves (tile_position)
            # -> full-array activity, HAM un-throttles to 2.4 GHz, and the
            # two 64-row matmuls stream concurrently (~194 ns per 512 cols).
            # ALL DMA rides the sync HWDGE ring, loads and stores interleaved
            # in pipeline order so the HBM stream never ping-pongs.
            def load_pair(pr):
                mfp = iop.tile([128, P2_CHUNK], bf16, tag="mf", name=f"mfp{pr}")
                nc.sync.dma_start(
                    out=mfp[0:64, :],
                    in_=mf[:, (2 * pr) * P2_CHUNK:(2 * pr + 1) * P2_CHUNK])
                nc.sync.dma_start(
                    out=mfp[64:128, :],
                    in_=mf[:, (2 * pr + 1) * P2_CHUNK:(2 * pr + 2) * P2_CHUNK])
                return mfp

            NPAIR = P2_NCHU // 2


            # prologue: pkt = (Wk^T @ sums^T) * (1/counts) + bk, cast bf16,
            # replicated onto both partition halves for the two row-groups.
            # (normalizing by counts commutes with the channel contraction)
            pro = psp.tile([NUM_MASKS, 2 * P2_TILE], f32, tag="big")
            pkraw = pro[0:KD, 0:NL]
            nc.tensor.matmul(pkraw, lhsT=wk_t, rhs=pt_t[0:C, :],
                             start=True, stop=True)
            pknorm = constp.tile([KD, NUM_MASKS], f32)
            nc.vector.tensor_tensor(out=pknorm[:], in0=pro[0:KD, 1:NL],
                                    in1=cn_t, op=mybir.AluOpType.mult)
            pkt_sb = constp.tile([KD, NUM_MASKS], f32)
            nc.vector.tensor_scalar_add(out=pkt_sb[:], in0=pknorm[:], scalar1=bk_t)
            pkt2 = constp.tile([128, NUM_MASKS], bf16)
            nc.vector.tensor_copy(out=pkt2[0:KD, :], in_=pkt_sb[:])
            nc.sync.dma_start(out=pkt2[KD:128, :], in_=pkt2[0:KD, :])

            # big bmm: out[100, MSH] = PK^T.T @ mask_features
            # Each psum tile [100, 2048] = even-chunk 1024 cols || odd-chunk
            # 1024 cols; ACT drains the even half, DVE the odd, concurrently.
            mfps = {pr: load_pair(pr) for pr in range(3)}
            for pr in range(NPAIR):
                mfp = mfps.pop(pr)
                ob = obp.tile([NUM_MASKS, 2 * P2_CHUNK], bf16, tag="ob", name=f"ob{pr}")
                for t in range(P2_CHUNK // P2_TILE):
                    ps = psp.tile([NUM_MASKS, 2 * P2_TILE], f32, tag="big")
                    for j in range(P2_TILE // 512):
                        col = t * P2_TILE + j * 512
                        nc.tensor.matmul(
                            ps[:, j * 512:(j + 1) * 512],
                            lhsT=pkt2[0:64, :],
                            rhs=mfp[0:64, col:col + 512],
                            start=True, stop=True, tile_position=(0, 0))
                        nc.tensor.matmul(
                            ps[:, P2_TILE + j * 512:P2_TILE + (j + 1) * 512],
                            lhsT=pkt2[64:128, :],
                            rhs=mfp[64:128, col:col + 512],
                            start=True, stop=True, tile_position=(64, 0))
                    nc.scalar.copy(
                        out=ob[:, t * P2_TILE:(t + 1) * P2_TILE],
                        in_=ps[:, 0:P2_TILE])
                    nc.vector.tensor_copy(
                        out=ob[:, P2_CHUNK + t * P2_TILE:P2_CHUNK + (t + 1) * P2_TILE],
                        in_=ps[:, P2_TILE:2 * P2_TILE])
                # one merged 3.2MB store per pair, alternating between the
                # gpsimd SWDGE ring and the scalar HWDGE ring so neither
                # ring's per-store overhead paces the pipeline
                eng = nc.gpsimd if pr % 2 == 0 else nc.scalar
                eng.dma_start(
                    out=om[:, (2 * pr) * P2_CHUNK:(2 * pr + 2) * P2_CHUNK],
                    in_=ob[:])
                if pr + 3 < NPAIR:
                    mfps[pr + 3] = load_pair(pr + 3)
    _split_excess_waits(nc)
    return nc


def _get_state():
    if not _STATE:
        _STATE["nc1"] = _build_phase1()
        _STATE["nc2"] = _build_phase2()
    return _STATE


def _run(nc, in_maps, tag):
    import os

    from concourse.bass_utils import run_bass_kernel_spmd

    trace = PROFILE is not None
    kw = {}
    tdir = os.environ.get("BASS_TRACE_DIR")
    if tdir:
        kw["tmpdir"] = os.path.join(tdir, tag)
        os.makedirs(kw["tmpdir"], exist_ok=True)
    res = run_bass_kernel_spmd(nc, in_maps, list(range(NCORES)), trace=trace, **kw)
    if PROFILE is not None:
        PROFILE[tag] = res.exec_time_ns
    return res.results


def kernel(features, mask_features, Wk, bk, init_masks):
    features = np.asarray(features, dtype=np.float32)
    mask_features = np.asarray(mask_features, dtype=np.float32)
    Wk = np.ascontiguousarray(np.asarray(Wk, dtype=np.float32))
    bk = np.asarray(bk, dtype=np.float32)
    init_masks = np.asarray(init_masks)

    st = _get_state()

    # ---- host-side sharding / layout prep ----
    feat = features.reshape(B, C, M)
    ftau = np.empty((B, M, 65), np.float32)
    ftau[:, :, :C] = feat.transpose(0, 2, 1)
    ftau[:, :, C] = 1.0
    # GPS_CHUNKS use the ACT bump one-hot (weight 1.125): pre-divide
    ftau.reshape(B, NSH, P1_NCH, 128 * P1_SUB, 65)[:, :, list(GPS_CHUNKS)] *= (1.0 / C0_BF)
    ftau = ftau.astype(BF16)
    labf = init_masks.reshape(B, M).astype(BF16)
    iota = np.ascontiguousarray(np.broadcast_to(
        np.arange(NLP, dtype=BF16)[None, None, :],
        (128, P1_SUB, NLP)).reshape(128, P1_SUB * NLP))

    in_maps1 = []
    for b in range(B):
        for s in range(NSH):
            sl = slice(s * MSH, (s + 1) * MSH)
            labr = labf[b, sl].reshape(P1_NCH, 128, P1_SUB).transpose(1, 0, 2)
            lab8 = np.ascontiguousarray(np.broadcast_to(
                labr[:, :, :, None], (128, P1_NCH, P1_SUB, 8))
                .reshape(128, P1_NCH * P1_SUB * 8))
            in_maps1.append({
                "ft": ftau[b, sl].reshape(P1_NCH, 128, P1_SUB * 65),
                "lab8": lab8,
                "labs": np.ascontiguousarray(labr.reshape(128, P1_NCH * P1_SUB)),
                "iota": iota,
            })
    r1 = _run(st["nc1"], in_maps1, "phase1")

    # combine shard partials per batch + count reciprocal (tiny glue)
    parts = np.stack([r["partials"] for r in r1]).reshape(B, NSH, 65, NLP).sum(axis=1)
    parts = parts[:, :, :NL]                              # drop pad labels
    cntr = 1.0 / np.maximum(parts[:, 64, 1:NL], 1.0)      # [B, 100]

    # pack sums^T / 1/counts / Wk / bk into one [128, 266] f32 tensor
    pcb = np.zeros((B, 128, 266), np.float32)
    pcb[:, 0:65, 0:NL] = parts
    pcb[:, 0:KD, NL:NL + NUM_MASKS] = cntr[:, None, :]
    pcb[:, 0:C, 201:201 + KD] = Wk[None]
    pcb[:, 0:KD, 265] = bk[None]

    mfr = mask_features.reshape(B, C, M).astype(BF16)
    in_maps2 = []
    for b in range(B):
        for s in range(NSH):
            sl = slice(s * MSH, (s + 1) * MSH)
            in_maps2.append({
                "pcb": pcb[b],
                "mf": mfr[b, :, sl],
            })
    r2 = _run(st["nc2"], in_maps2, "phase2")

    out = np.empty((B, NUM_MASKS, M), np.float32)
    for i in range(NCORES):
        b, s = divmod(i, NSH)
        out[b, :, s * MSH:(s + 1) * MSH] = r2[i]["om"]  # bf16 -> f32 upcast
    return out.reshape(B, NUM_MASKS, D, H, W)


# revision 34
# speedup vs baseline: 1.1020x; 1.0325x over previous
"""Trainium2 Bass kernel for nn_InstDecoder (segment_reduce + bmm).

Computation (reference semantics):
  1. Per batch b: per-label masked mean of features over voxels
     inst[b, n, c] = mean_{v: labels[b,v]==n+1} features[b, c, v]   (labels 1..100)
  2. pred_kernel = inst @ Wk + bk                                   [B, 100, 64]
  3. pred_masks = pred_kernel @ mask_features.reshape(B, 64, M)     [B, 100, M]

Sharding: data-parallel over B (=2) x 4-way split of the flattened voxel axis
M = D*H*W = 524288 -> 8 cores, each owning a [*, 131072] voxel shard.

Phase 1 (device): per-core partial (sums, counts) over its shard via
one-hot(labels) matmuls accumulated in PSUM -> [65, 104] partials (labels
padded to 104 = 13*8; cols 101..103 are never matched so they stay zero).
The one-hot is built s-major ([128, SUB, 104], label innermost) so the PE
streams contiguous rhs slices. Production is split across engines:
  - most chunks: DVE tensor_tensor is_equal in 2x mode. The label operand is
    pre-repeated 8x on the host (lab8) so BOTH inputs walk innermost step-1
    runs: out[p, s, hi, lo] = (lab8[p, s, lo] == iota104[s, hi, lo]).
  - GPS_CHUNKS: GPSIMD subtract (d = lab - iota) then ACT
    Derivative_Erf(64*d) -- an exact bump: bf16(1.1283792)=1.125 at d==0,
    exactly 0 elsewhere. The 1.125 scale is cancelled by pre-scaling those
    chunks' features (and ones column) by 1/1.125 on the host.
Host: sum partials across the 4 shards of each batch + reciprocal of counts
(tiny glue).

Phase 2 (device): pred_kernel^T = (Wk^T @ sums^T) * (1/counts) + bk  [64,100]
(normalization commutes with the channel contraction), cast bf16, then the
big bmm over the mask_features shard -> [100, 131072] bf16.
PSUM staged as 4 x [100, 1024] 2-bank tiles; each drained by concurrent
ACT/DVE half-copies so banks free in ~0.75us and the PE never idles (stays
at the warm 2.4 GHz clock). mf loads ride the sync HWDGE ring, per-chunk
output stores the gpsimd SWDGE ring, so the streams never block each other.

Features are pre-transposed on the host during sharding (with a ones column
appended for the counts row) so the device kernels need no on-chip transpose
of the bulk data.
"""

import ml_dtypes
import numpy as np

BF16 = ml_dtypes.bfloat16

# ---- problem constants (hardcoded per contract) ----
B = 2
C = 64
KD = 64
D, H, W = 8, 256, 256
M = D * H * W            # 524288 voxels per batch
NUM_MASKS = 100
NL = NUM_MASKS + 1       # labels 0..100 (0 dropped at the end)
NLP = 104                # padded label count (13 * 8) for the 2x-mode one-hot
NSH = 4                  # voxel shards per batch
MSH = M // NSH           # 131072 voxels per core
NCORES = B * NSH

# phase-1 tiling: chunks of [128 partitions, SUB voxel-columns]
P1_SUB = 64
P1_NCH = MSH // (128 * P1_SUB)   # 16 chunks of 8192 voxels
# chunks built by gps-subtract + ACT-bump (rest: DVE is_equal), interleaved
# so production completion order tracks the PE's in-order consumption
GPS_CHUNKS = ()
C0_BF = 1.125                    # bf16(Derivative_Erf table value at 0)

# phase-2 tiling: voxel chunks per DMA load; [100, 2048] psum tiles holding
# one 1024-col group of the even chunk + the same cols of the odd chunk
P2_CHUNK = 8192
P2_NCHU = MSH // P2_CHUNK        # 16
P2_TILE = 1024
P2_NT = P2_CHUNK // P2_TILE      # 8

_STATE = {}

# test.py can set this to a dict; per-phase HW exec times (ns) get stored.
PROFILE = None


def _tile_context(nc):
    """TileContext whose kernel-tail drain splits its semaphore waits into
    one wait_ge instruction each — this container's walrus rejects CTRL
    instructions carrying more than a couple of sync waits."""
    import concourse.tile as tile
    from concourse.vector_clock import ScopedClock

    class _SplitDrainTC(tile.TileContext):
        def _drain_and_barrier(self, tick_clock, wait_clock):
            nc = self.nc
            drain_inst = nc.sync.drain()
            wait_clock.add_sem_waits(
                drain_inst.ins, ScopedClock({None: tick_clock.global_clock}))
            si = drain_inst.ins.sync_info
            waits = list(si.on_wait) if si and si.on_wait else []
            handles = {s.name: s for s in self.sems.allocated().values()}
            if waits:
                si.on_wait = []
                for w in waits:
                    nc.sync.wait_ge(handles[w.ant_name], w.wait_value)
            nc.all_engine_barrier()
            popped = nc._tile_sem_poison_stack.pop()
            assert popped is self._sem_poison
            nc.clear_and_free_semaphores(list(self.sems.allocated().values()))
            nc.all_engine_barrier()

    return _SplitDrainTC(nc)


def _split_excess_waits(nc, max_waits=1):
    """This container's walrus rejects instructions carrying more than
    ~2 semaphore waits. Move excess waits onto same-engine nops inserted
    immediately before the offending instruction (monotonic sems make
    this semantically equivalent)."""
    import bass_rust

    created = {}
    new_names = set()
    for bb in nc.main_func.blocks:
        for ins in bb.instructions:
            if ins.name in new_names:
                continue
            si = ins.sync_info
            if si and si.on_wait and len(si.on_wait) > max_waits:
                waits = list(si.on_wait)
                si.on_wait = waits[:max_waits]
                extra = waits[max_waits:]
                nops = []
                for k in range(0, len(extra), max_waits):
                    n = nc.engines[ins.engine].nop(nofuse=True)
                    n.ins.sync_info = bass_rust.SyncInfo(
                        on_wait=extra[k:k + max_waits], on_update=[])
                    nops.append(n.ins)
                    new_names.add(n.ins.name)
                created[ins.name] = nops
    if not created:
        return
    for bb in nc.main_func.blocks:
        out = []
        for ins in bb.instructions:
            if ins.name in new_names:
                continue
            if ins.name in created:
                out.extend(created[ins.name])
            out.append(ins)
        bb.instructions = out


def _build_phase1():
    import concourse.bass as bass
    import concourse.mybir as mybir
    import concourse.tile as tile

    f32 = mybir.dt.float32
    bf16 = mybir.dt.bfloat16
    SUB = P1_SUB
    nc = bass.Bass()
    ft = nc.declare_dram_parameter("ft", [P1_NCH, 128, SUB * 65], bf16, isOutput=False)
    lab8 = nc.declare_dram_parameter("lab8", [128, P1_NCH * SUB * 8], bf16, isOutput=False)
    labs = nc.declare_dram_parameter("labs", [128, P1_NCH * SUB], bf16, isOutput=False)
    iota = nc.declare_dram_parameter("iota", [128, SUB * NLP], bf16, isOutput=False)
    part = nc.declare_dram_parameter("partials", [65, NLP], f32, isOutput=True)

    with _tile_context(nc) as tc:
        with tc.tile_pool(name="const", bufs=1) as constp, \
             tc.tile_pool(name="io", bufs=3) as iop, \
             tc.tile_pool(name="ohd", bufs=4) as ohdp, \
             tc.tile_pool(name="ohg", bufs=2) as ohgp, \
             tc.tile_pool(name="df", bufs=2) as dfp, \
             tc.tile_pool(name="ps", bufs=1, space="PSUM") as psp, \
             tc.tile_pool(name="out", bufs=1) as outp:
            # constants first so the one-hot producers start early
            lab_t = constp.tile([128, P1_NCH * SUB * 8], bf16)
            nc.sync.dma_start(out=lab_t[:], in_=lab8[:])
            iota_t = constp.tile([128, SUB * NLP], bf16)
            nc.sync.dma_start(out=iota_t[:], in_=iota[:])
            if GPS_CHUNKS:
                # separate tiles per reader: DVE and gps hammering the same
                # tile costs DVE ~66% via SBUF bank conflicts
                labs_t = constp.tile([128, P1_NCH * SUB], bf16)
                nc.sync.dma_start(out=labs_t[:], in_=labs[:])
                iota_g = constp.tile([128, SUB * NLP], bf16)
                nc.sync.dma_start(out=iota_g[:], in_=iota[:])
            # issue every ft load up-front; the 3-slot ring self-throttles
            ftts = []
            for c in range(P1_NCH):
                ftt = iop.tile([128, SUB * 65], bf16, tag="ft")
                nc.sync.dma_start(out=ftt[:], in_=ft[c])
                ftts.append(ftt)
            acc = psp.tile([65, NLP], f32)
            iview = iota_t[:].rearrange("p (s h o) -> p s h o", h=13, o=8)
            gview = iota_g[:].rearrange("p (s l) -> p s l", l=NLP) if GPS_CHUNKS else None
            for c in range(P1_NCH):
                if c not in GPS_CHUNKS:
                    lab_sl = lab_t[:, c * SUB * 8:(c + 1) * SUB * 8] \
                        .rearrange("p (s o) -> p s o", o=8)
                    oht = ohdp.tile([128, SUB * NLP], bf16, tag="ohd")
                    nc.vector.tensor_tensor(
                        out=oht[:].rearrange("p (s h o) -> p s h o", h=13, o=8),
                        in0=lab_sl[:, :, None, :].broadcast_to([128, SUB, 13, 8]),
                        in1=iview,
                        op=mybir.AluOpType.is_equal,
                    )
                else:
                    labsl = labs_t[:, c * SUB:(c + 1) * SUB]
                    dt_ = dfp.tile([128, SUB * NLP], bf16, tag="d")
                    nc.gpsimd.tensor_tensor(
                        out=dt_[:].rearrange("p (s l) -> p s l", l=NLP),
                        in0=labsl[:, :, None].broadcast_to([128, SUB, NLP]),
                        in1=gview,
                        op=mybir.AluOpType.subtract,
                    )
                    oht = ohgp.tile([128, SUB * NLP], bf16, tag="ohg")
                    nc.scalar.activation(
                        out=oht[:], in_=dt_[:],
                        func=mybir.ActivationFunctionType.Derivative_Erf,
                        scale=64.0)
                for j in range(SUB):
                    nc.tensor.matmul(
                        acc[:],
                        lhsT=ftts[c][:, j * 65:(j + 1) * 65],
                        rhs=oht[:, j * NLP:(j + 1) * NLP],
                        start=(c == 0 and j == 0),
                        stop=(c == P1_NCH - 1 and j == SUB - 1),
                    )
            out_t = outp.tile([65, NLP], f32)
            nc.vector.tensor_copy(out=out_t[:], in_=acc[:])
            nc.sync.dma_start(out=part[:], in_=out_t[:])
    _split_excess_waits(nc)
    return nc


def _build_phase2():
    import concourse.bass as bass
    import concourse.mybir as mybir
    import concourse.tile as tile

    f32 = mybir.dt.float32
    bf16 = mybir.dt.bfloat16
    nc = bass.Bass()
    # packed constants: rows 0:65 sums^T [65,101]; cols 101:201 rows 0:64 are
    # 1/counts [64,100]; cols 201:265 rows 0:64 are Wk [64,64]; col 265 is bk
    pcb = nc.declare_dram_parameter("pcb", [128, 266], f32, isOutput=False)
    mf = nc.declare_dram_parameter("mf", [C, MSH], bf16, isOutput=False)
    om = nc.declare_dram_parameter("om", [NUM_MASKS, MSH], bf16, isOutput=True)

    with _tile_context(nc) as tc:
        with tc.tile_pool(name="const", bufs=1) as constp, \
             tc.tile_pool(name="io", bufs=3) as iop, \
             tc.tile_pool(name="ob", bufs=3) as obp, \
             tc.tile_pool(name="ps", bufs=2, space="PSUM") as psp:
            pcb_t = constp.tile([128, 266], f32)
            nc.sync.dma_start(out=pcb_t[:], in_=pcb[:])
            pt_t = pcb_t[0:65, 0:NL]
            cn_t = pcb_t[0:KD, NL:NL + NUM_MASKS]
            wk_t = pcb_t[0:C, 201:201 + KD]
            bk_t = pcb_t[0:KD, 265:266]

            # chunk PAIRS: even chunk on partitions 0:64, odd on 64:128, so
            # interleaved matmuls occupy both PE row-halves (tile_position)
            # -> full-array activity, HAM un-throttles to 2.4 GHz, and the
            # two 64-row matmuls stream concurrently (~194 ns per 512 cols).
            # ALL DMA rides the sync HWDGE ring, loads and stores interleaved
            # in pipeline order so the HBM stream never ping-pongs.
            def load_pair(pr):
                mfp = iop.tile([128, P2_CHUNK], bf16, tag="mf", name=f"mfp{pr}")
                nc.sync.dma_start(
                    out=mfp[0:64, :],
                    in_=mf[:, (2 * pr) * P2_CHUNK:(2 * pr + 1) * P2_CHUNK])
                nc.sync.dma_start(
                    out=mfp[64:128, :],
                    in_=mf[:, (2 * pr + 1) * P2_CHUNK:(2 * pr + 2) * P2_CHUNK])
                return mfp

            NPAIR = P2_NCHU // 2


            # prologue: pkt = (Wk^T @ sums^T) * (1/counts) + bk, cast bf16,
            # replicated onto both partition halves for the two row-groups.
            # (normalizing by counts commutes with the channel contraction)
            pro = psp.tile([NUM_MASKS, 2 * P2_TILE], f32, tag="big")
            pkraw = pro[0:KD, 0:NL]
            nc.tensor.matmul(pkraw, lhsT=wk_t, rhs=pt_t[0:C, :],
                             start=True, stop=True)
            pknorm = constp.tile([KD, NUM_MASKS], f32)
            nc.vector.tensor_tensor(out=pknorm[:], in0=pro[0:KD, 1:NL],
                                    in1=cn_t, op=mybir.AluOpType.mult)
            pkt_sb = constp.tile([KD, NUM_MASKS], f32)
            nc.vector.tensor_scalar_add(out=pkt_sb[:], in0=pknorm[:], scalar1=bk_t)
            pkt2 = constp.tile([128, NUM_MASKS], bf16)
            nc.vector.tensor_copy(out=pkt2[0:KD, :], in_=pkt_sb[:])
            nc.sync.dma_start(out=pkt2[KD:128, :], in_=pkt2[0:KD, :])

            # big bmm: out[100, MSH] = PK^T.T @ mask_features
            # Each psum tile [100, 2048] = even-chunk 1024 cols || odd-chunk
            # 1024 cols; ACT drains the even half, DVE the odd, concurrently.
            mfps = {pr: load_pair(pr) for pr in range(3)}
            for pr in range(NPAIR):
                mfp = mfps.pop(pr)
                ob = obp.tile([NUM_MASKS, 2 * P2_CHUNK], bf16, tag="ob", name=f"ob{pr}")
                for t in range(P2_CHUNK // P2_TILE):
                    ps = psp.tile([NUM_MASKS, 2 * P2_TILE], f32, tag="big")
                    for j in range(P2_TILE // 512):
                        col = t * P2_TILE + j * 512
                        nc.tensor.matmul(
                            ps[:, j * 512:(j + 1) * 512],
                            lhsT=pkt2[0:64, :],
                            rhs=mfp[0:64, col:col + 512],
                            start=True, stop=True, tile_position=(0, 0))
                        nc.tensor.matmul(
                            ps[:, P2_TILE + j * 512:P2_TILE + (j + 1) * 512],
                            lhsT=pkt2[64:128, :],
                            rhs=mfp[64:128, col:col + 512],
                            start=True, stop=True, tile_position=(64, 0))
                    nc.scalar.copy(
                        out=ob[:, t * P2_TILE:(t + 1) * P2_TILE],
                        in_=ps[:, 0:P2_TILE])
                    nc.vector.tensor_copy(
                        out=ob[:, P2_CHUNK + t * P2_TILE:P2_CHUNK + (t + 1) * P2_TILE],
                        in_=ps[:, P2_TILE:2 * P2_TILE])
                # one merged 3.2MB store per pair on the otherwise-idle
                # gpsimd ring so store latency overlaps the sync-ring loads
                nc.gpsimd.dma_start(
                    out=om[:, (2 * pr) * P2_CHUNK:(2 * pr + 2) * P2_CHUNK],
                    in_=ob[:])
                if pr + 3 < NPAIR:
                    mfps[pr + 3] = load_pair(pr + 3)
    _split_excess_waits(nc)
    return nc


def _get_state():
    if not _STATE:
        _STATE["nc1"] = _build_phase1()
        _STATE["nc2"] = _build_phase2()
    return _STATE


def _run(nc, in_maps, tag):
    import os

    from concourse.bass_utils import run_bass_kernel_spmd

    trace = PROFILE is not None
    kw = {}
    tdir = os.environ.get("BASS_TRACE_DIR")
    if tdir:
        kw["tmpdir"] = os.path.join(tdir, tag)
        os.makedirs(kw["tmpdir"], exist_ok=True)
    res = run_bass_kernel_spmd(nc, in_maps, list(range(NCORES)), trace=trace, **kw)
    if PROFILE is not None:
        PROFILE[tag] = res.exec_time_ns
    return res.results


def kernel(features, mask_features, Wk, bk, init_masks):
    features = np.asarray(features, dtype=np.float32)
    mask_features = np.asarray(mask_features, dtype=np.float32)
    Wk = np.ascontiguousarray(np.asarray(Wk, dtype=np.float32))
    bk = np.asarray(bk, dtype=np.float32)
    init_masks = np.asarray(init_masks)

    st = _get_state()

    # ---- host-side sharding / layout prep ----
    feat = features.reshape(B, C, M)
    ftau = np.empty((B, M, 65), np.float32)
    ftau[:, :, :C] = feat.transpose(0, 2, 1)
    ftau[:, :, C] = 1.0
    # GPS_CHUNKS use the ACT bump one-hot (weight 1.125): pre-divide
    ftau.reshape(B, NSH, P1_NCH, 128 * P1_SUB, 65)[:, :, list(GPS_CHUNKS)] *= (1.0 / C0_BF)
    ftau = ftau.astype(BF16)
    labf = init_masks.reshape(B, M).astype(BF16)
    iota = np.ascontiguousarray(np.broadcast_to(
        np.arange(NLP, dtype=BF16)[None, None, :],
        (128, P1_SUB, NLP)).reshape(128, P1_SUB * NLP))

    in_maps1 = []
    for b in range(B):
        for s in range(NSH):
            sl = slice(s * MSH, (s + 1) * MSH)
            labr = labf[b, sl].reshape(P1_NCH, 128, P1_SUB).transpose(1, 0, 2)
            lab8 = np.ascontiguousarray(np.broadcast_to(
                labr[:, :, :, None], (128, P1_NCH, P1_SUB, 8))
                .reshape(128, P1_NCH * P1_SUB * 8))
            in_maps1.append({
                "ft": ftau[b, sl].reshape(P1_NCH, 128, P1_SUB * 65),
                "lab8": lab8,
                "labs": np.ascontiguousarray(labr.reshape(128, P1_NCH * P1_SUB)),
                "iota": iota,
            })
    r1 = _run(st["nc1"], in_maps1, "phase1")

    # combine shard partials per batch + count reciprocal (tiny glue)
    parts = np.stack([r["partials"] for r in r1]).reshape(B, NSH, 65, NLP).sum(axis=1)
    parts = parts[:, :, :NL]                              # drop pad labels
    cntr = 1.0 / np.maximum(parts[:, 64, 1:NL], 1.0)      # [B, 100]

    # pack sums^T / 1/counts / Wk / bk into one [128, 266] f32 tensor
    pcb = np.zeros((B, 128, 266), np.float32)
    pcb[:, 0:65, 0:NL] = parts
    pcb[:, 0:KD, NL:NL + NUM_MASKS] = cntr[:, None, :]
    pcb[:, 0:C, 201:201 + KD] = Wk[None]
    pcb[:, 0:KD, 265] = bk[None]

    mfr = mask_features.reshape(B, C, M).astype(BF16)
    in_maps2 = []
    for b in range(B):
        for s in range(NSH):
            sl = slice(s * MSH, (s + 1) * MSH)
            in_maps2.append({
                "pcb": pcb[b],
                "mf": mfr[b, :, sl],
            })
    r2 = _run(st["nc2"], in_maps2, "phase2")

    out = np.empty((B, NUM_MASKS, M), np.float32)
    for i in range(NCORES):
        b, s = divmod(i, NSH)
        out[b, :, s * MSH:(s + 1) * MSH] = r2[i]["om"]  # bf16 -> f32 upcast
    return out.reshape(B, NUM_MASKS, D, H, W)
